# revision 1
# baseline (speedup 1.0000x reference)
"""Trainium2 Bass kernel for GaussianFPSPooling.

Pipeline (per batch element, one NeuronCore):
  1. Farthest-point sampling over N=100000 3-D points, K=256 iterations,
     fully SBUF-resident.  Arithmetic replicates the jax-CPU reference
     bit-exactly ((x-px)^2 + (y-py)^2) + (z-pz)^2, f32, left-assoc, min
     accumulate, first-index argmax) so the selected indices match.
  2. Indirect-DMA gather of the 256 selected feature rows from HBM.
  3. PE transpose + matmul with W (f32) + bias.

Distribution: data-parallel over the batch (B=4) across 8 cores; cores
c and c+4 run the same batch (c % 4), host reads cores 0-3.
"""

import sys

if "/opt/trn_rl_repo" not in sys.path:
    sys.path.insert(0, "/opt/trn_rl_repo")

import numpy as np

import concourse.bacc as bacc
import concourse.bass as bass
import concourse.bass_isa as bass_isa
import concourse.mybir as mybir
from concourse import tile
from concourse.bass_utils import run_bass_kernel_spmd

F32 = mybir.dt.float32
I32 = mybir.dt.int32
Alu = mybir.AluOpType
Act = mybir.ActivationFunctionType

# problem sizes (hardcoded per contract)
B = 4
N = 100000
D_IN = 128
D_OUT = 256
K = 256
P = 128               # partitions
BIGI = float(1 << 20)  # index-encoding base: stores BIGI - idx (exact in f32)


def _ceil_div(a, b):
    return (a + b - 1) // b


def build_fps_kernel(n=N, k=K, d_in=D_IN, d_out=D_OUT, with_linear=True):
    """Build the Bass program. Returns (nc, C) with C = cols per partition."""
    C = _ceil_div(n, P)
    npad = P * C

    nc = bacc.Bacc("TRN2", target_bir_lowering=False)

    # ---- DRAM I/O ----
    xs_d = nc.dram_tensor("xs", [P, C], F32, kind="ExternalInput")
    ys_d = nc.dram_tensor("ys", [P, C], F32, kind="ExternalInput")
    zs_d = nc.dram_tensor("zs", [P, C], F32, kind="ExternalInput")
    g2_d = nc.dram_tensor("g2", [P, C], F32, kind="ExternalInput")
    dists_d = nc.dram_tensor("dists0", [P, C], F32, kind="ExternalInput")
    pt0_d = nc.dram_tensor("pt0", [P, 4], F32, kind="ExternalInput")
    idx_d = nc.dram_tensor("idx_out", [1, k], F32, kind="ExternalOutput")
    if with_linear:
        feat_d = nc.dram_tensor("feat", [n, d_in], F32, kind="ExternalInput")
        w_d = nc.dram_tensor("w", [d_in, d_out], F32, kind="ExternalInput")
        brow_d = nc.dram_tensor("brow", [1, d_out], F32, kind="ExternalInput")
        ones1_d = nc.dram_tensor("ones1", [1, P], F32, kind="ExternalInput")
        ident_d = nc.dram_tensor("ident", [P, P], F32, kind="ExternalInput")
        out_d = nc.dram_tensor("out", [k, d_out], F32, kind="ExternalOutput")

    kg = k // P if with_linear else 0  # gather column groups
    if with_linear:
        assert k % P == 0

    # position of iteration-k index inside idxraw (so a plain [1,k]->[P,kg]
    # SBUF->SBUF DMA lands index of sample k at partition k%P, col k//P)
    if with_linear:
        pos = [(kk % P) * kg + (kk // P) for kk in range(k)]
    else:
        pos = list(range(k))

    with tile.TileContext(nc) as tc:
        with (
            tc.tile_pool(name="const", bufs=1) as cp,
            tc.tile_pool(name="loop", bufs=2) as lp,
            tc.tile_pool(name="psum", bufs=2, space="PSUM") as pp,
        ):
            xs = cp.tile([P, C], F32, tag="xs")
            ys = cp.tile([P, C], F32, tag="ys")
            zs = cp.tile([P, C], F32, tag="zs")
            g2 = cp.tile([P, C], F32, tag="g2")
            dists = cp.tile([P, C], F32, tag="dists")
            pt0 = cp.tile([P, 4], F32, tag="pt0")
            idxraw = cp.tile([1, k], F32, tag="idxraw")

            nc.sync.dma_start(xs[:], xs_d[:])
            nc.sync.dma_start(ys[:], ys_d[:])
            nc.sync.dma_start(zs[:], zs_d[:])
            nc.sync.dma_start(g2[:], g2_d[:])
            nc.sync.dma_start(dists[:], dists_d[:])
            nc.sync.dma_start(pt0[:], pt0_d[:])
            nc.vector.memset(idxraw[:], BIGI)  # sample 0 is point 0

            pt = pt0
            for it in range(k - 1):
                px = pt[:, 0:1]
                py = pt[:, 1:2]
                pz = pt[:, 2:3]
                # d = ((x-px)^2 + (y-py)^2) + (z-pz)^2, bit-exact f32
                t1 = lp.tile([P, C], F32, tag="t1")
                nc.scalar.activation(t1[:], xs[:], Act.Square, bias=px, scale=-1.0)
                t2 = lp.tile([P, C], F32, tag="t2")
                nc.scalar.activation(t2[:], ys[:], Act.Square, bias=py, scale=-1.0)
                t3 = lp.tile([P, C], F32, tag="t3")
                nc.scalar.activation(t3[:], zs[:], Act.Square, bias=pz, scale=-1.0)
                s = lp.tile([P, C], F32, tag="s")
                nc.vector.tensor_tensor(s[:], t1[:], t2[:], op=Alu.add)
                nc.vector.tensor_tensor(s[:], s[:], t3[:], op=Alu.add)
                # dists = min(dists, d); permax = rowwise max of new dists
                # (tensor_tensor_reduce would fuse these but crashes this
                # runtime, so keep them split)
                permax = lp.tile([P, 1], F32, tag="permax")
                nc.vector.tensor_tensor(dists[:], dists[:], s[:], op=Alu.min)
                nc.vector.reduce_max(permax[:], dists[:], axis=mybir.AxisListType.X)
                gmax = lp.tile([P, 1], F32, tag="gmax")
                nc.gpsimd.partition_all_reduce(
                    gmax[:], permax[:], channels=P, reduce_op=bass_isa.ReduceOp.max
                )
                # encode argmax as max over (dists==gmax)*(BIGI-idx)
                mi = lp.tile([P, C], F32, tag="mi")
                nc.vector.scalar_tensor_tensor(
                    mi[:], in0=dists[:], scalar=gmax[:], in1=g2[:],
                    op0=Alu.is_equal, op1=Alu.mult,
                )
                permax2 = lp.tile([P, 1], F32, tag="permax2")
                nc.vector.reduce_max(permax2[:], mi[:], axis=mybir.AxisListType.X)
                is2 = lp.tile([P, 1], F32, tag="is2")
                nc.gpsimd.partition_all_reduce(
                    is2[:], permax2[:], channels=P, reduce_op=bass_isa.ReduceOp.max
                )
                # record BIGI - idx (decoded after the loop)
                nc.scalar.copy(idxraw[0:1, pos[it + 1] : pos[it + 1] + 1],
                               is2[0:1, 0:1])
                # extract winner coords: one-hot (g2==is2) dot each plane
                ptn = lp.tile([P, 4], F32, tag="ptn")
                junk = lp.tile([P, C], F32, tag="junk")
                nc.vector.scalar_tensor_tensor(
                    junk[:], in0=g2[:], scalar=is2[:], in1=xs[:],
                    op0=Alu.is_equal, op1=Alu.mult, accum_out=ptn[:, 0:1],
                )
                nc.vector.scalar_tensor_tensor(
                    junk[:], in0=g2[:], scalar=is2[:], in1=ys[:],
                    op0=Alu.is_equal, op1=Alu.mult, accum_out=ptn[:, 1:2],
                )
                nc.vector.scalar_tensor_tensor(
                    junk[:], in0=g2[:], scalar=is2[:], in1=zs[:],
                    op0=Alu.is_equal, op1=Alu.mult, accum_out=ptn[:, 2:3],
                )
                ptb = lp.tile([P, 4], F32, tag="ptb")
                nc.gpsimd.partition_all_reduce(
                    ptb[:, 0:3], ptn[:, 0:3], channels=P,
                    reduce_op=bass_isa.ReduceOp.add,
                )
                pt = ptb

            # decode indices: idx = BIGI - idxraw
            idxf = cp.tile([1, k], F32, tag="idxf")
            nc.vector.tensor_scalar(
                idxf[:], idxraw[:], -1.0, BIGI, op0=Alu.mult, op1=Alu.add
            )
            nc.sync.dma_start(idx_d[:], idxf[:])

            if with_linear:
                w_sb = cp.tile([d_in, d_out], F32, tag="w")
                brow = cp.tile([1, d_out], F32, tag="brow")
                ones1 = cp.tile([1, P], F32, tag="ones1")
                ident = cp.tile([P, P], F32, tag="ident")
                nc.sync.dma_start(w_sb[:], w_d[:])
                nc.sync.dma_start(brow[:], brow_d[:])
                nc.sync.dma_start(ones1[:], ones1_d[:])
                nc.sync.dma_start(ident[:], ident_d[:])

                idxi = cp.tile([1, k], I32, tag="idxi")
                nc.vector.tensor_copy(idxi[:], idxf[:])
                gidx = cp.tile([P, kg], I32, tag="gidx")
                nc.sync.dma_start(gidx[:], idxi[:])  # relayout [1,k]->[P,kg]

                for j in range(kg):
                    gath = cp.tile([P, d_in], F32, tag=f"gath{j}")
                    nc.gpsimd.indirect_dma_start(
                        out=gath[:],
                        out_offset=None,
                        in_=feat_d[:],
                        in_offset=bass.IndirectOffsetOnAxis(
                            ap=gidx[:, j : j + 1], axis=0
                        ),
                    )
                    tp_ps = pp.tile([P, P], F32, tag="tp")
                    nc.tensor.transpose(tp_ps[:], gath[:], ident[:])
                    lhsT = cp.tile([P, P], F32, tag=f"lhsT{j}")
                    nc.vector.tensor_copy(lhsT[:], tp_ps[:])
                    out_ps = pp.tile([P, d_out], F32, tag="outps")
                    nc.tensor.matmul(
                        out_ps[:], lhsT=lhsT[:], rhs=w_sb[:], start=True, stop=False
                    )
                    nc.tensor.matmul(
                        out_ps[:], lhsT=ones1[:], rhs=brow[:], start=False, stop=True
                    )
                    outt = cp.tile([P, d_out], F32, tag=f"outt{j}")
                    nc.vector.tensor_copy(outt[:], out_ps[:])
                    nc.sync.dma_start(out_d[j * P : (j + 1) * P, :], outt[:])

    nc.compile()
    return nc, C


def make_core_inputs(means_b, features_b=None, W=None, bvec=None,
                     n=N, k=K, with_linear=True):
    """Host-side layout for one batch element."""
    C = _ceil_div(n, P)
    npad = P * C
    m = np.asarray(means_b, np.float32)
    planes = np.zeros((npad, 3), np.float32)
    planes[:n] = m
    d0 = np.full(npad, -1.0, np.float32)
    d0[:n] = np.inf
    g2 = np.zeros(npad, np.float32)
    g2[:n] = BIGI - np.arange(n, dtype=np.float32)
    pt0 = np.zeros((P, 4), np.float32)
    pt0[:, 0:3] = m[0]
    d = {
        "xs": planes[:, 0].reshape(P, C).copy(),
        "ys": planes[:, 1].reshape(P, C).copy(),
        "zs": planes[:, 2].reshape(P, C).copy(),
        "g2": g2.reshape(P, C).copy(),
        "dists0": d0.reshape(P, C).copy(),
        "pt0": pt0,
    }
    if with_linear:
        d["feat"] = np.ascontiguousarray(features_b, dtype=np.float32)
        d["w"] = np.ascontiguousarray(W, dtype=np.float32)
        d["brow"] = np.ascontiguousarray(bvec, dtype=np.float32).reshape(1, -1)
        d["ones1"] = np.ones((1, P), np.float32)
        d["ident"] = np.eye(P, dtype=np.float32)
    return d


_CACHE = {}


def _get_kernel():
    if "nc" not in _CACHE:
        _CACHE["nc"] = build_fps_kernel()[0]
    return _CACHE["nc"]


def kernel(features, means, W, b, trace=False):
    features = np.asarray(features, np.float32)
    means = np.asarray(means, np.float32)
    W = np.asarray(W, np.float32)
    b = np.asarray(b, np.float32)

    nc = _get_kernel()
    in_maps = []
    for c in range(8):
        bb = c % B
        in_maps.append(make_core_inputs(means[bb], features[bb], W, b))
    import time as _time

    t0 = _time.time()
    res = run_bass_kernel_spmd(nc, in_maps, core_ids=list(range(8)), trace=trace)
    _CACHE["last_run_s"] = _time.time() - t0
    out = np.stack([res.results[bb]["out"] for bb in range(B)], axis=0)
    _CACHE["last_results"] = res
    return out


if __name__ == "__main__":
    ins = dict(np.load("/tmp/inputs.npz"))
    out = kernel(**ins)
    print("out", out.shape, out.dtype)



# revision 3
# speedup vs baseline: 48.0445x; 48.0445x over previous
"""Trainium2 Bass kernel for GaussianFPSPooling.

Reference computation: farthest-point-sample K=256 of N=100000 3-D points
per batch element (B=4), gather the selected feature rows [256,128], then
a Linear to [256,256].

The previous version shipped the full features tensor (51MB x 8 cores =
410MB) through the axon tunnel every call so an on-device indirect DMA
could gather the 256 selected rows; that transfer was ~90% of the wall
time.  This version never puts features on the device:

  Phase A (device, cores 0-3, one batch element each): farthest-point
      sampling over the packed coords tensor xyz [128, 3*782] f32
      (1.2MB/core — the only phase-A input).  The index-encoding plane,
      initial distances, and the coords of point 0 are generated on
      device (iota + memset + affine_select + one-hot extract).  The
      arithmetic replicates the jax-CPU reference bit-exactly
      (((x-px)^2 + (y-py)^2) + (z-pz)^2, f32, min accumulate,
      first-index argmax), so the selected indices match exactly.
  Host: gather the 256 selected feature rows per batch (131KB) and
      transpose to the matmul lhsT layout.
  Phase B (device, cores 0-3): [256,128] @ [128,256] + b on the PE
      array, inputs packed as sw [128, 512] = [sT | W] plus the bias row.

Both phases run through a cached jax.jit(shard_map(_bass_exec)) runner —
the same PJRT/axon execution path bass_utils.run_bass_kernel_spmd takes
under axon, minus its per-call jit re-trace (which cost ~0.6s/call).
Data-parallel over batch per the sharding hint; with B=4 batch elements
a 4-device mesh moves half the bytes of an 8-device mesh of duplicated
pairs and measures faster, so cores 4-7 idle.
"""

import sys

if "/opt/trn_rl_repo" not in sys.path:
    sys.path.insert(0, "/opt/trn_rl_repo")

import numpy as np

import concourse.bacc as bacc
import concourse.bass_isa as bass_isa
import concourse.mybir as mybir
from concourse import tile

F32 = mybir.dt.float32
Alu = mybir.AluOpType
Act = mybir.ActivationFunctionType

# problem sizes (hardcoded per contract)
B = 4
N = 100000
D_IN = 128
D_OUT = 256
K = 256
P = 128
BIGI = float(1 << 20)  # index-encoding base: stores BIGI - idx (exact in f32)
C = (N + P - 1) // P   # 782 cols per partition
NPAD = P * C
DINF = 3.0e38          # +inf surrogate for the running min-distance
NCORES = 4             # one core per batch element


def build_fps_nc():
    nc = bacc.Bacc("TRN2", target_bir_lowering=False)

    xyz_d = nc.dram_tensor("xyz", [P, 3 * C], F32, kind="ExternalInput")
    idx_d = nc.dram_tensor("idx_out", [1, K], F32, kind="ExternalOutput")

    with tile.TileContext(nc) as tc:
        with (
            tc.tile_pool(name="const", bufs=1) as cp,
            tc.tile_pool(name="loop", bufs=2) as lp,
        ):
            xyz = cp.tile([P, 3 * C], F32, tag="xyz")
            nc.sync.dma_start(xyz[:], xyz_d[:])
            xs = xyz[:, 0:C]
            ys = xyz[:, C : 2 * C]
            zs = xyz[:, 2 * C : 3 * C]

            # g2[p,c] = BIGI - (p*C + c): flat index encoded so larger value
            # means smaller index (first-index argmax tiebreak), exact in f32
            g2i = cp.tile([P, C], mybir.dt.int32, tag="g2i")
            nc.gpsimd.iota(
                g2i[:], [[-1, C]], base=int(BIGI), channel_multiplier=-C
            )
            g2 = cp.tile([P, C], F32, tag="g2")
            nc.vector.tensor_copy(g2[:], g2i[:])

            # running min-distance: +inf surrogate, padding lanes (flat index
            # >= N) pinned to -1 so they never win the argmax
            dists = cp.tile([P, C], F32, tag="dists")
            nc.vector.memset(dists[:], DINF)
            nc.gpsimd.affine_select(
                dists[:], dists[:], [[-1, C]], Alu.is_ge, -1.0,
                base=N - 1, channel_multiplier=-C,
            )

            idxraw = cp.tile([1, K], F32, tag="idxraw")
            nc.vector.memset(idxraw[:], BIGI)  # sample 0 is point 0

            def extract_pt(dst, sel, pool):
                # winner coords via one-hot (g2 == sel) dot each plane
                ptn = pool.tile([P, 4], F32, tag="ptn")
                junk = pool.tile([P, C], F32, tag="junk")
                for ax, plane in enumerate((xs, ys, zs)):
                    nc.vector.scalar_tensor_tensor(
                        junk[:], in0=g2[:], scalar=sel[:], in1=plane,
                        op0=Alu.is_equal, op1=Alu.mult,
                        accum_out=ptn[:, ax : ax + 1],
                    )
                dstb = pool.tile([P, 4], F32, tag="dstb")
                nc.gpsimd.partition_all_reduce(
                    dstb[:, 0:3], ptn[:, 0:3], channels=P,
                    reduce_op=bass_isa.ReduceOp.add,
                )
                return dstb

            # coords of point 0 (g2 == BIGI only at flat index 0)
            is0 = cp.tile([P, 1], F32, tag="is0")
            nc.vector.memset(is0[:], BIGI)
            pt = extract_pt(None, is0, cp)

            for it in range(K - 1):
                px = pt[:, 0:1]
                py = pt[:, 1:2]
                pz = pt[:, 2:3]
                # d = ((x-px)^2 + (y-py)^2) + (z-pz)^2, bit-exact f32
                t1 = lp.tile([P, C], F32, tag="t1")
                nc.scalar.activation(t1[:], xs, Act.Square, bias=px, scale=-1.0)
                t2 = lp.tile([P, C], F32, tag="t2")
                nc.scalar.activation(t2[:], ys, Act.Square, bias=py, scale=-1.0)
                t3 = lp.tile([P, C], F32, tag="t3")
                nc.scalar.activation(t3[:], zs, Act.Square, bias=pz, scale=-1.0)
                s = lp.tile([P, C], F32, tag="s")
                nc.vector.tensor_tensor(s[:], t1[:], t2[:], op=Alu.add)
                nc.vector.tensor_tensor(s[:], s[:], t3[:], op=Alu.add)
                # dists = min(dists, d); then global max + first-index argmax
                permax = lp.tile([P, 1], F32, tag="permax")
                nc.vector.tensor_tensor(dists[:], dists[:], s[:], op=Alu.min)
                nc.vector.reduce_max(permax[:], dists[:], axis=mybir.AxisListType.X)
                gmax = lp.tile([P, 1], F32, tag="gmax")
                nc.gpsimd.partition_all_reduce(
                    gmax[:], permax[:], channels=P, reduce_op=bass_isa.ReduceOp.max
                )
                mi = lp.tile([P, C], F32, tag="mi")
                nc.vector.scalar_tensor_tensor(
                    mi[:], in0=dists[:], scalar=gmax[:], in1=g2[:],
                    op0=Alu.is_equal, op1=Alu.mult,
                )
                permax2 = lp.tile([P, 1], F32, tag="permax2")
                nc.vector.reduce_max(permax2[:], mi[:], axis=mybir.AxisListType.X)
                is2 = lp.tile([P, 1], F32, tag="is2")
                nc.gpsimd.partition_all_reduce(
                    is2[:], permax2[:], channels=P, reduce_op=bass_isa.ReduceOp.max
                )
                # record BIGI - idx (decoded after the loop)
                nc.scalar.copy(idxraw[0:1, it + 1 : it + 2], is2[0:1, 0:1])
                if it < K - 2:  # last iteration's winner coords are never used
                    pt = extract_pt(None, is2, lp)

            # decode indices: idx = BIGI - idxraw
            idxf = cp.tile([1, K], F32, tag="idxf")
            nc.vector.tensor_scalar(
                idxf[:], idxraw[:], -1.0, BIGI, op0=Alu.mult, op1=Alu.add
            )
            nc.sync.dma_start(idx_d[:], idxf[:])

    nc.compile()
    return nc


def build_linear_nc():
    nc = bacc.Bacc("TRN2", target_bir_lowering=False)

    sw_d = nc.dram_tensor("sw", [P, K + D_OUT], F32, kind="ExternalInput")
    brow_d = nc.dram_tensor("brow", [1, D_OUT], F32, kind="ExternalInput")
    out_d = nc.dram_tensor("out", [K, D_OUT], F32, kind="ExternalOutput")

    kg = K // P
    with tile.TileContext(nc) as tc:
        with (
            tc.tile_pool(name="const", bufs=1) as cp,
            tc.tile_pool(name="psum", bufs=2, space="PSUM") as pp,
        ):
            sw = cp.tile([P, K + D_OUT], F32, tag="sw")
            brow = cp.tile([1, D_OUT], F32, tag="brow")
            ones1 = cp.tile([1, P], F32, tag="ones1")
            nc.sync.dma_start(sw[:], sw_d[:])
            nc.sync.dma_start(brow[:], brow_d[:])
            nc.vector.memset(ones1[:], 1.0)
            w_sb = sw[:, K : K + D_OUT]

            for j in range(kg):
                out_ps = pp.tile([P, D_OUT], F32, tag="outps")
                nc.tensor.matmul(
                    out_ps[:], lhsT=sw[:, j * P : (j + 1) * P], rhs=w_sb,
                    start=True, stop=False,
                )
                # bias add: ones(128) outer b accumulated into the same bank
                nc.tensor.matmul(
                    out_ps[:], lhsT=ones1[:], rhs=brow[:], start=False, stop=True
                )
                outt = cp.tile([P, D_OUT], F32, tag=f"outt{j}")
                nc.vector.tensor_copy(outt[:], out_ps[:])
                nc.sync.dma_start(out_d[j * P : (j + 1) * P, :], outt[:])

    nc.compile()
    return nc


def make_runner(nc, n_cores):
    """Cached-jit SPMD runner: the axon/PJRT path of run_bass_kernel_spmd
    (bass2jax._bass_exec_p under jit+shard_map), with the jitted callable
    built once and reused across kernel() calls."""
    import jax
    from jax.sharding import Mesh, PartitionSpec
    from jax.experimental.shard_map import shard_map
    from concourse import bass2jax

    bass2jax.install_neuronx_cc_hook()
    partition_name = nc.partition_id_tensor.name if nc.partition_id_tensor else None
    in_names, out_names, out_avals = [], [], []
    for alloc in nc.m.functions[0].allocations:
        if not isinstance(alloc, mybir.MemoryLocationSet):
            continue
        name = alloc.memorylocations[0].name
        if alloc.kind == "ExternalInput":
            if name != partition_name:
                in_names.append(name)
        elif alloc.kind == "ExternalOutput":
            out_names.append(name)
            out_avals.append(jax.core.ShapedArray(
                tuple(alloc.tensor_shape), mybir.dt.np(alloc.dtype)))
    n_params = len(in_names)
    n_outs = len(out_avals)
    all_names = in_names + out_names
    if partition_name is not None:
        all_names.append(partition_name)
    donate = tuple(range(n_params, n_params + n_outs))

    def _body(*args):
        operands = list(args)
        if partition_name is not None:
            operands.append(bass2jax.partition_id_tensor())
        outs = bass2jax._bass_exec_p.bind(
            *operands,
            out_avals=tuple(out_avals),
            in_names=tuple(all_names),
            out_names=tuple(out_names),
            lowering_input_output_aliases=(),
            sim_require_finite=True,
            sim_require_nnan=True,
            nc=nc,
        )
        return tuple(outs)

    devices = jax.devices()[:n_cores]
    mesh = Mesh(np.asarray(devices), ("core",))
    in_specs = (PartitionSpec("core"),) * (n_params + n_outs)
    out_specs = (PartitionSpec("core"),) * n_outs
    sharded = jax.jit(
        shard_map(_body, mesh=mesh, in_specs=in_specs, out_specs=out_specs,
                  check_rep=False),
        donate_argnums=donate, keep_unused=True)

    def run(in_maps):
        assert len(in_maps) == n_cores
        per_core = [[np.asarray(m[name]) for name in in_names] for m in in_maps]
        concat_in = [
            np.concatenate([per_core[c][i] for c in range(n_cores)], axis=0)
            for i in range(n_params)
        ]
        # PJRT allocates custom_call results uninit; donate zero buffers so
        # kernels that don't write every element see zeros (run_neff parity)
        concat_zeros = [np.zeros((n_cores * a.shape[0], *a.shape[1:]), a.dtype)
                        for a in out_avals]
        out_arrs = sharded(*concat_in, *concat_zeros)
        return [
            {name: np.asarray(out_arrs[i]).reshape(n_cores, *out_avals[i].shape)[c]
             for i, name in enumerate(out_names)}
            for c in range(n_cores)
        ]

    return run


def pack_xyz(means_b):
    """[N,3] coords -> [P, 3*C]: plane-major, each plane row-major [P,C]."""
    m = np.asarray(means_b, np.float32)
    buf = np.zeros((3, NPAD), np.float32)
    buf[:, :N] = m.T
    return buf.reshape(3, P, C).transpose(1, 0, 2).reshape(P, 3 * C).copy()


_CACHE = {}


def _get_runners():
    if "runners" not in _CACHE:
        _CACHE["runners"] = (
            make_runner(build_fps_nc(), NCORES),
            make_runner(build_linear_nc(), NCORES),
        )
    return _CACHE["runners"]


def kernel(features, means, W, b):
    import time as _time

    features = np.asarray(features, np.float32)
    means = np.asarray(means, np.float32)
    W = np.asarray(W, np.float32)
    bvec = np.asarray(b, np.float32)

    run_a, run_b = _get_runners()

    t0 = _time.time()
    in_maps_a = [{"xyz": pack_xyz(means[c % B])} for c in range(NCORES)]
    res_a = run_a(in_maps_a)
    idx = np.stack([res_a[bb]["idx_out"][0].astype(np.int64) for bb in range(B)])

    brow = bvec.reshape(1, -1)
    in_maps_b = []
    for c in range(NCORES):
        bb = c % B
        sw = np.empty((P, K + D_OUT), np.float32)
        sw[:, :K] = features[bb][idx[bb]].T
        sw[:, K:] = W
        in_maps_b.append({"sw": sw, "brow": brow})
    res_b = run_b(in_maps_b)
    _CACHE["last_run_s"] = _time.time() - t0

    return np.stack([res_b[bb]["out"] for bb in range(B)], axis=0)


if __name__ == "__main__":
    ins = dict(np.load("/tmp/inputs.npz"))
    out = kernel(**ins)
    print("out", out.shape, out.dtype)
